# revision 29
# baseline (speedup 1.0000x reference)
"""GQA kernel for trn2, 8 cores: DP over batch (2) x TP over kv-head groups (4).

Each core computes, for its (batch b, kv-group g):
  - qkv projection for its 4 q-heads + 1 kv-head (q pre-scaled by 1/sqrt(dk))
  - RoPE on q/k
  - full (non-causal) attention for the 4 q-heads vs its kv-head
  - partial out-projection with its 2048 rows of W_out
Host sums the 4 per-group partials per batch and adds bias.

Matmul operands are bf16 (PE runs fp32 at 1/4 rate; bf16 is full rate).
Accumulation and softmax statistics stay fp32.

Perf notes:
  - all inputs are pre-arranged on the host into the exact SBUF layout
    (partition-dim first) so every DMA moves large contiguous packets
  - x is pre-transposed on the host; no PE cycles are spent transposing
  - softmax denominator off the tensor engine: vector adds (one behind
    each exp) + gpsimd partition all-reduce + vector reciprocal
  - score matmuls of the next pair are interleaved into the PV matmul
    stream (1 per 4) so the scalar engine's exp rate never stalls the PE
  - W_out cached in SBUF once (loaded during the B->C transition)
  - warmup matmuls ramp the PE clock out of its low p-state while the
    first input DMAs are in flight

Self-contained: hardcodes all shapes. kernel(**inputs) -> np.ndarray.
"""

import math
from contextlib import ExitStack

import numpy as np
import ml_dtypes

import concourse.bass as bass
import concourse.bass_isa as bass_isa
import concourse.bacc as bacc
import concourse.tile as tile
import concourse.mybir as mybir
from concourse.bass_utils import run_bass_kernel_spmd

F32 = mybir.dt.float32
F32R = mybir.dt.float32r
BF16 = mybir.dt.bfloat16
L = 2048          # sequence length
D = 2048          # d_model
DK = 128          # head dim (q/k)
DV = 512          # head dim (v)
NHQ = 4           # q heads per core
NI = 4            # query chunks of 512
NJ = 16           # key chunks of 128
NDCH = 16         # d_model chunks of 128

_NC_CACHE = {}


def build_nc():
    if "nc" in _NC_CACHE:
        return _NC_CACHE["nc"]
    nc = bacc.Bacc("TRN2", target_bir_lowering=False, debug=False)

    # all inputs pre-arranged to SBUF layout on the host
    xT_d = nc.dram_tensor("xt", [NI, 128, NDCH, 512], BF16, kind="ExternalInput")
    wqk_d = nc.dram_tensor("wqk", [5, 128, NDCH, 128], BF16, kind="ExternalInput")
    wv_d = nc.dram_tensor("wv", [128, NDCH, DV], BF16, kind="ExternalInput")
    wo_d = nc.dram_tensor("wo", [128, NDCH, D], BF16, kind="ExternalInput")
    cos_d = nc.dram_tensor("cost", [DK, L], F32, kind="ExternalInput")
    sin_d = nc.dram_tensor("sint", [DK, L], F32, kind="ExternalInput")
    out_d = nc.dram_tensor("out", [L, D], F32, kind="ExternalOutput")

    EXP = mybir.ActivationFunctionType.Exp

    with ExitStack() as ctx:
        tc = ctx.enter_context(tile.TileContext(nc))
        # long-lived pools
        persist = ctx.enter_context(tc.tile_pool(name="persist", bufs=1))
        pexp = ctx.enter_context(tc.tile_pool(name="pexp", bufs=2))
        pden = ctx.enter_context(tc.tile_pool(name="pden", bufs=2))
        pctx = ctx.enter_context(tc.tile_pool(name="pctx", bufs=1))
        psS = ctx.enter_context(tc.tile_pool(name="psS", bufs=5, space="PSUM"))
        psA = ctx.enter_context(tc.tile_pool(name="psA", bufs=3, space="PSUM"))

        qT = persist.tile([128, NHQ, L], BF16)      # [dk, h, pos]
        kT = persist.tile([128, L], BF16)           # [dk, pos]
        v_sb = persist.tile([128, NJ, DV], BF16)    # [key_in_chunk, key_chunk, e]
        ones_f = persist.tile([128, 128], F32R)     # all-ones: denom reduce
        ones_f32 = persist.tile([128, 128], F32)
        nc.vector.memset(ones_f32, 1.0)
        nc.vector.tensor_copy(out=ones_f, in_=ones_f32)

        # warmup: ramp the PE out of its low p-state while input DMAs fly
        warm = persist.tile([128, 512], BF16)
        nc.vector.memset(warm, 0.0)
        for w in range(22):
            wps = psS.tile([128, 512], F32, tag="stream")
            nc.tensor.matmul(wps, lhsT=warm[:, 0:128], rhs=warm)

        # ---- S/exp/denominator step queue (absorbed into other segments) ----
        sq = []
        # denominator tail steps (partition-reduce matmul + reciprocal),
        # drained late in the next PV segment, after the vector adds finish
        tailq = []

        def queue_s(i, h):
            """Queue the 16 score-matmul+exp+denom steps for pair (i, h)."""
            isl = slice(i * 512, (i + 1) * 512)
            expS = pexp.tile([128, NJ, 512], BF16, tag="expS")
            acc = pden.tile([128, 512], F32R, tag="dacc")
            rb = pden.tile([128, 512], F32, tag="rb")

            def mk(j):
                def f():
                    ps = psS.tile([128, 512], F32, tag="stream")
                    nc.tensor.matmul(ps, lhsT=kT[:, j * 128:(j + 1) * 128],
                                     rhs=qT[:, h, isl])
                    nc.scalar.activation(out=expS[:, j, :], in_=ps, func=EXP)
                    if j == 1:
                        nc.vector.tensor_add(acc, expS[:, 0, :], expS[:, 1, :])
                    elif j > 1:
                        nc.vector.tensor_add(acc, acc, expS[:, j, :])
                return f

            def tail():
                # all-ones stationary: one full-rate matmul both sums over
                # partitions and replicates the denominator to all partitions
                dps = psS.tile([128, 512], F32, tag="stream")
                nc.tensor.matmul(dps, lhsT=ones_f, rhs=acc)
                nc.vector.reciprocal_approx_fast(out=rb, in_=dps)

            sq.extend(mk(j) for j in range(NJ))
            tailq.append(tail)
            return expS, rb

        def drain(n=1):
            for _ in range(n):
                if sq:
                    sq.pop(0)()

        def drain_tail():
            while tailq:
                tailq.pop(0)()

        # ---------------- Phase B: qkv projection + rope ----------------
        with tc.tile_pool(name="pb1", bufs=1) as pb1, \
             tc.tile_pool(name="pbx", bufs=2) as pbx, \
             tc.tile_pool(name="pb2", bufs=2) as pb2:
            wqk_sb = pb1.tile([128, 5, NDCH, 128], BF16)
            wv_sb = pb1.tile([128, NDCH, DV], BF16)
            cosT = pb1.tile([128, L], F32)
            sinT = pb1.tile([128, L], F32)

            xTs = [pbx.tile([128, NDCH, 512], BF16, tag="xT", name=f"xT{i}")
                   for i in range(NI)]
            # DMA queues: sync (HW) = xT; scalar (HW) = wqk; gpsimd (SW) =
            # rope tables + wv. Issue order == first-use order.
            for tq in range(4):
                nc.sync.dma_start(out=xTs[0][:, 4 * tq:4 * tq + 4, :],
                                  in_=xT_d.ap()[0][:, 4 * tq:4 * tq + 4, :])
            for c in range(5):
                nc.scalar.dma_start(out=wqk_sb[:, c], in_=wqk_d.ap()[c])
            nc.gpsimd.dma_start(out=cosT[:, 0:512], in_=cos_d.ap()[:, 0:512])
            nc.gpsimd.dma_start(out=sinT[:, 0:512], in_=sin_d.ap()[:, 0:512])
            nc.gpsimd.dma_start(out=cosT[:, 512:], in_=cos_d.ap()[:, 512:])
            nc.gpsimd.dma_start(out=sinT[:, 512:], in_=sin_d.ap()[:, 512:])
            nc.gpsimd.dma_start(out=wv_sb, in_=wv_d.ap())
            for i in range(1, NI):
                nc.sync.dma_start(out=xTs[i], in_=xT_d.ap()[i])

            for i in range(NI):
                isl = slice(i * 512, (i + 1) * 512)
                xT = xTs[i]
                # q/k projection + rope (c = 0..3 q heads, c = 4 is k)
                for c in range(5):
                    ps = psA.tile([128, 512], F32, tag="acc")
                    for t in range(NDCH):
                        nc.tensor.matmul(
                            ps, lhsT=wqk_sb[:, c, t, :],
                            rhs=xT[:, t, :],
                            start=(t == 0), stop=(t == NDCH - 1))
                    dest = qT[:, c, isl] if c < NHQ else kT[:, isl]
                    cs = cosT[:, isl]
                    sn = sinT[:, isl]
                    tmp = pb2.tile([128, 512], F32, tag="rope")
                    nc.vector.tensor_mul(tmp[0:64, :], ps[64:128, :], sn[0:64, :])
                    nc.vector.tensor_mul(tmp[64:128, :], ps[0:64, :], sn[64:128, :])
                    tmp2 = pb2.tile([128, 512], F32, tag="rope2")
                    nc.vector.tensor_mul(tmp2, ps, cs)
                    nc.vector.tensor_sub(dest[0:64, :], tmp2[0:64, :], tmp[0:64, :])
                    nc.vector.tensor_add(dest[64:128, :], tmp2[64:128, :],
                                         tmp[64:128, :])

                # v projection; the i==3 block absorbs the score matmuls of
                # the first attention pair (kT is complete at that point)
                if i == NI - 1:
                    first_handle = queue_s(0, 0)
                for lsub in range(4):
                    ps = psA.tile([128, 512], F32, tag="acc")
                    for t in range(NDCH):
                        nc.tensor.matmul(
                            ps, lhsT=xT[:, t, lsub * 128:(lsub + 1) * 128],
                            rhs=wv_sb[:, t, :],
                            start=(t == 0), stop=(t == NDCH - 1))
                        if i == NI - 1 and (lsub * NDCH + t) % 2 == 1:
                            drain(1)
                    nc.scalar.copy(out=v_sb[:, i * 4 + lsub, :], in_=ps)
                if i == NI - 1:
                    drain_tail()

        # ---------------- Phase C+D: attention + fused out-projection -------
        with tc.tile_pool(name="pc2", bufs=2) as pc2, \
             tc.tile_pool(name="pcw", bufs=1) as pcw:
            # W_out cached for all 4 i-blocks; the DMA overlaps the first
            # attention pairs (its SBUF region frees as phase B retires)
            wo_sb = pcw.tile([128, NDCH, D], BF16)
            nc.gpsimd.dma_start(out=wo_sb, in_=wo_d.ap())

            ctxTs = {}

            def emit_pv(i, h, expS, rb):
                ctxT = pctx.tile([128, 4, 512], BF16, tag=f"ctx{h}")
                for ec in range(4):
                    ps = psA.tile([128, 512], F32, tag="acc")
                    for j in range(NJ):
                        nc.tensor.matmul(
                            ps, lhsT=v_sb[:, j, ec * 128:(ec + 1) * 128],
                            rhs=expS[:, j, :],
                            start=(j == 0), stop=(j == NJ - 1))
                        # front-loaded (1 per 2): the denominator adds of the
                        # next pair resolve mid-segment
                        if (ec * NJ + j) % 2 == 1:
                            drain(1)
                        # its reduce+reciprocal tail goes late, after the
                        # vector adds have certainly finished
                        if ec == 3 and j == 8:
                            drain_tail()
                    nc.vector.tensor_mul(ctxT[:, ec, :], ps, rb)
                ctxTs[h] = ctxT

            def emit_outproj(i):
                for dm in range(4):
                    for lsub in range(4):
                        ps = psA.tile([128, 512], F32, tag="acc")
                        for h in range(NHQ):
                            for ec in range(4):
                                t = h * 4 + ec
                                nc.tensor.matmul(
                                    ps,
                                    lhsT=ctxTs[h][:, ec,
                                                  lsub * 128:(lsub + 1) * 128],
                                    rhs=wo_sb[:, t, dm * 512:(dm + 1) * 512],
                                    start=(t == 0), stop=(t == 15))
                        ost = pc2.tile([128, 512], F32, tag="ost")
                        nc.scalar.copy(out=ost, in_=ps)
                        l0 = i * 512 + lsub * 128
                        nc.sync.dma_start(
                            out=out_d.ap()[l0:l0 + 128,
                                           dm * 512:(dm + 1) * 512],
                            in_=ost)

            pairs = [(i, h) for i in range(NI) for h in range(NHQ)]
            handles = {(0, 0): first_handle}
            for k, (i, h) in enumerate(pairs):
                if k + 1 < len(pairs):
                    handles[pairs[k + 1]] = queue_s(*pairs[k + 1])
                emit_pv(i, h, *handles.pop((i, h)))
                if h == NHQ - 1:
                    emit_outproj(i)
            assert not sq

    nc.compile()
    _NC_CACHE["nc"] = nc
    return nc


def make_core_inputs(x, W_attn, W_out):
    """Split full inputs into 8 per-core input maps (core = b*4 + g).

    All tensors are pre-arranged into the exact SBUF layout (partition
    dim first) so device DMAs are large contiguous packets.
    """
    Q_DIM = 2048
    K_DIM = 512
    scale = np.float32(1.0 / math.sqrt(DK))
    bf = ml_dtypes.bfloat16

    # rope tables, mirroring the fp32 reference computation
    inv_freq = (np.float32(1.0) /
                (np.float32(10000.0) **
                 (np.arange(0, DK, 2, dtype=np.float32) / np.float32(DK))))
    freqs = np.arange(L, dtype=np.float32)[:, None] * inv_freq[None, :]  # [L,64]
    ang = np.concatenate([freqs, freqs], axis=-1)  # [L, 128]
    cosT = np.ascontiguousarray(np.cos(ang).T.astype(np.float32))  # [128, L]
    sinT = np.ascontiguousarray(np.sin(ang).T.astype(np.float32))

    # x^T -> [NI, 128, NDCH, 512]: block i, partition p, chunk t, q
    xTb = []
    for b in range(2):
        xt = x[b].T.astype(bf)                       # [D, L]
        xt = xt.reshape(NDCH, 128, NI, 512).transpose(2, 1, 0, 3)
        xTb.append(np.ascontiguousarray(xt))

    in_maps = []
    for core in range(8):
        b, g = divmod(core, 4)
        wq = (W_attn[:, 512 * g:512 * (g + 1)] * scale)
        wk = W_attn[:, Q_DIM + 128 * g:Q_DIM + 128 * (g + 1)]
        wqk = np.concatenate([wq, wk], axis=1).astype(bf)        # [D, 640]
        wqk = np.ascontiguousarray(
            wqk.reshape(NDCH, 128, 5, 128).transpose(2, 1, 0, 3))
        wv = W_attn[:, Q_DIM + K_DIM + 512 * g:
                    Q_DIM + K_DIM + 512 * (g + 1)].astype(bf)    # [D, 512]
        wv = np.ascontiguousarray(wv.reshape(NDCH, 128, DV).transpose(1, 0, 2))
        wo = W_out[2048 * g:2048 * (g + 1), :].astype(bf)        # [2048, D]
        wo = np.ascontiguousarray(wo.reshape(NDCH, 128, D).transpose(1, 0, 2))
        in_maps.append({
            "xt": xTb[b],
            "wqk": wqk,
            "wv": wv,
            "wo": wo,
            "cost": cosT,
            "sint": sinT,
        })
    return in_maps


def kernel(x, W_attn, W_out, b_out, _trace=False, _trace_cores=None):
    x = np.asarray(x)
    W_attn = np.asarray(W_attn)
    W_out = np.asarray(W_out)
    b_out = np.asarray(b_out)
    nc = build_nc()
    in_maps = make_core_inputs(x, W_attn, W_out)
    res = run_bass_kernel_spmd(
        nc, in_maps, core_ids=list(range(8)),
        trace=_trace, trace_cores=_trace_cores)
    parts = [res.results[c]["out"] for c in range(8)]
    out = np.empty((2, L, D), dtype=np.float32)
    for b in range(2):
        acc = parts[4 * b].astype(np.float32)
        for g in range(1, 4):
            acc = acc + parts[4 * b + g]
        out[b] = acc + b_out[None, :].astype(np.float32)
    if _trace:
        kernel._last_results = res
    return out


# revision 30
# speedup vs baseline: 1.0156x; 1.0156x over previous
"""GQA kernel for trn2, 8 cores: DP over batch (2) x TP over kv-head groups (4).

Each core computes, for its (batch b, kv-group g):
  - qkv projection for its 4 q-heads + 1 kv-head (q pre-scaled by 1/sqrt(dk))
  - RoPE on q/k
  - full (non-causal) attention for the 4 q-heads vs its kv-head
  - partial out-projection with its 2048 rows of W_out
Host sums the 4 per-group partials per batch and adds bias.

Matmul operands are bf16 (PE runs fp32 at 1/4 rate; bf16 is full rate).
Accumulation and softmax statistics stay fp32.

Perf notes:
  - all inputs are pre-arranged on the host into the exact SBUF layout
    (partition-dim first) so every DMA moves large contiguous packets
  - x is pre-transposed on the host; no PE cycles are spent transposing
  - softmax denominator off the tensor engine: vector adds (one behind
    each exp) + gpsimd partition all-reduce + vector reciprocal
  - score matmuls of the next pair are interleaved into the PV matmul
    stream (1 per 4) so the scalar engine's exp rate never stalls the PE
  - W_out cached in SBUF once (loaded during the B->C transition)
  - warmup matmuls ramp the PE clock out of its low p-state while the
    first input DMAs are in flight

Self-contained: hardcodes all shapes. kernel(**inputs) -> np.ndarray.
"""

import math
from contextlib import ExitStack

import numpy as np
import ml_dtypes

import concourse.bass as bass
import concourse.bass_isa as bass_isa
import concourse.bacc as bacc
import concourse.tile as tile
import concourse.mybir as mybir
from concourse.bass_utils import run_bass_kernel_spmd

F32 = mybir.dt.float32
F32R = mybir.dt.float32r
BF16 = mybir.dt.bfloat16
L = 2048          # sequence length
D = 2048          # d_model
DK = 128          # head dim (q/k)
DV = 512          # head dim (v)
NHQ = 4           # q heads per core
NI = 4            # query chunks of 512
NJ = 16           # key chunks of 128
NDCH = 16         # d_model chunks of 128

_NC_CACHE = {}


def build_nc():
    if "nc" in _NC_CACHE:
        return _NC_CACHE["nc"]
    nc = bacc.Bacc("TRN2", target_bir_lowering=False, debug=False)

    # all inputs pre-arranged to SBUF layout on the host
    xT_d = nc.dram_tensor("xt", [NI, 128, NDCH, 512], BF16, kind="ExternalInput")
    wqk_d = nc.dram_tensor("wqk", [5, 128, NDCH, 128], BF16, kind="ExternalInput")
    wv_d = nc.dram_tensor("wv", [128, NDCH, DV], BF16, kind="ExternalInput")
    wo_d = nc.dram_tensor("wo", [128, NDCH, D], BF16, kind="ExternalInput")
    cos_d = nc.dram_tensor("cost", [DK, L], F32, kind="ExternalInput")
    sin_d = nc.dram_tensor("sint", [DK, L], F32, kind="ExternalInput")
    out_d = nc.dram_tensor("out", [L, D], F32, kind="ExternalOutput")

    EXP = mybir.ActivationFunctionType.Exp

    with ExitStack() as ctx:
        tc = ctx.enter_context(tile.TileContext(nc))
        # long-lived pools
        persist = ctx.enter_context(tc.tile_pool(name="persist", bufs=1))
        pexp = ctx.enter_context(tc.tile_pool(name="pexp", bufs=2))
        pden = ctx.enter_context(tc.tile_pool(name="pden", bufs=2))
        pctx = ctx.enter_context(tc.tile_pool(name="pctx", bufs=1))
        psS = ctx.enter_context(tc.tile_pool(name="psS", bufs=5, space="PSUM"))
        psA = ctx.enter_context(tc.tile_pool(name="psA", bufs=3, space="PSUM"))

        qT = persist.tile([128, NHQ, L], BF16)      # [dk, h, pos]
        kT = persist.tile([128, L], BF16)           # [dk, pos]
        v_sb = persist.tile([128, NJ, DV], BF16)    # [key_in_chunk, key_chunk, e]
        ones_f = persist.tile([128, 128], F32R)     # all-ones: denom reduce
        ones_f32 = persist.tile([128, 128], F32)
        nc.vector.memset(ones_f32, 1.0)
        nc.vector.tensor_copy(out=ones_f, in_=ones_f32)

        # warmup: ramp the PE out of its low p-state while input DMAs fly
        warm = persist.tile([128, 512], BF16)
        nc.vector.memset(warm, 0.0)
        for w in range(12):
            wps = psS.tile([128, 512], F32, tag="stream")
            nc.tensor.matmul(wps, lhsT=warm[:, 0:128], rhs=warm)

        # ---- S/exp/denominator step queue (absorbed into other segments) ----
        sq = []
        # denominator tail steps (partition-reduce matmul + reciprocal),
        # drained late in the next PV segment, after the vector adds finish
        tailq = []

        def queue_s(i, h):
            """Queue the 16 score-matmul+exp+denom steps for pair (i, h)."""
            isl = slice(i * 512, (i + 1) * 512)
            expS = pexp.tile([128, NJ, 512], BF16, tag="expS")
            acc = pden.tile([128, 512], F32R, tag="dacc")
            rb = pden.tile([128, 512], F32, tag="rb")

            def mk(j):
                def f():
                    ps = psS.tile([128, 512], F32, tag="stream")
                    nc.tensor.matmul(ps, lhsT=kT[:, j * 128:(j + 1) * 128],
                                     rhs=qT[:, h, isl])
                    nc.scalar.activation(out=expS[:, j, :], in_=ps, func=EXP)
                    if j == 1:
                        nc.vector.tensor_add(acc, expS[:, 0, :], expS[:, 1, :])
                    elif j > 1:
                        nc.vector.tensor_add(acc, acc, expS[:, j, :])
                return f

            def tail():
                # all-ones stationary: one full-rate matmul both sums over
                # partitions and replicates the denominator to all partitions
                dps = psS.tile([128, 512], F32, tag="stream")
                nc.tensor.matmul(dps, lhsT=ones_f, rhs=acc)
                nc.vector.reciprocal_approx_fast(out=rb, in_=dps)

            sq.extend(mk(j) for j in range(NJ))
            tailq.append(tail)
            return expS, rb

        def drain(n=1):
            for _ in range(n):
                if sq:
                    sq.pop(0)()

        def drain_tail():
            while tailq:
                tailq.pop(0)()

        # ---------------- Phase B: qkv projection + rope ----------------
        with tc.tile_pool(name="pb1", bufs=1) as pb1, \
             tc.tile_pool(name="pbx", bufs=2) as pbx, \
             tc.tile_pool(name="pb2", bufs=2) as pb2:
            wqk_sb = pb1.tile([128, 5, NDCH, 128], BF16)
            wv_sb = pb1.tile([128, NDCH, DV], BF16)
            cosT = pb1.tile([128, L], F32)
            sinT = pb1.tile([128, L], F32)

            xTs = [pbx.tile([128, NDCH, 512], BF16, tag="xT", name=f"xT{i}")
                   for i in range(NI)]
            # DMA issue order == first-use order. sync queue: xT chunks.
            # gpsimd queue: weights + rope tables.
            for tq in range(4):
                nc.sync.dma_start(out=xTs[0][:, 4 * tq:4 * tq + 4, :],
                                  in_=xT_d.ap()[0][:, 4 * tq:4 * tq + 4, :])
            nc.gpsimd.dma_start(out=wqk_sb[:, 0], in_=wqk_d.ap()[0])
            nc.gpsimd.dma_start(out=wqk_sb[:, 1], in_=wqk_d.ap()[1])
            nc.gpsimd.dma_start(out=cosT[:, 0:512], in_=cos_d.ap()[:, 0:512])
            nc.gpsimd.dma_start(out=sinT[:, 0:512], in_=sin_d.ap()[:, 0:512])
            for c in range(2, 5):
                nc.gpsimd.dma_start(out=wqk_sb[:, c], in_=wqk_d.ap()[c])
            nc.gpsimd.dma_start(out=cosT[:, 512:], in_=cos_d.ap()[:, 512:])
            nc.gpsimd.dma_start(out=sinT[:, 512:], in_=sin_d.ap()[:, 512:])
            nc.gpsimd.dma_start(out=wv_sb, in_=wv_d.ap())
            for i in range(1, NI):
                nc.sync.dma_start(out=xTs[i], in_=xT_d.ap()[i])

            for i in range(NI):
                isl = slice(i * 512, (i + 1) * 512)
                xT = xTs[i]
                # q/k projection + rope (c = 0..3 q heads, c = 4 is k)
                for c in range(5):
                    ps = psA.tile([128, 512], F32, tag="acc")
                    for t in range(NDCH):
                        nc.tensor.matmul(
                            ps, lhsT=wqk_sb[:, c, t, :],
                            rhs=xT[:, t, :],
                            start=(t == 0), stop=(t == NDCH - 1))
                    dest = qT[:, c, isl] if c < NHQ else kT[:, isl]
                    cs = cosT[:, isl]
                    sn = sinT[:, isl]
                    tmp = pb2.tile([128, 512], F32, tag="rope")
                    nc.vector.tensor_mul(tmp[0:64, :], ps[64:128, :], sn[0:64, :])
                    nc.vector.tensor_mul(tmp[64:128, :], ps[0:64, :], sn[64:128, :])
                    tmp2 = pb2.tile([128, 512], F32, tag="rope2")
                    nc.vector.tensor_mul(tmp2, ps, cs)
                    nc.vector.tensor_sub(dest[0:64, :], tmp2[0:64, :], tmp[0:64, :])
                    nc.vector.tensor_add(dest[64:128, :], tmp2[64:128, :],
                                         tmp[64:128, :])

                # v projection; the i==3 block absorbs the score matmuls of
                # the first attention pair (kT is complete at that point)
                if i == NI - 1:
                    first_handle = queue_s(0, 0)
                for lsub in range(4):
                    ps = psA.tile([128, 512], F32, tag="acc")
                    for t in range(NDCH):
                        nc.tensor.matmul(
                            ps, lhsT=xT[:, t, lsub * 128:(lsub + 1) * 128],
                            rhs=wv_sb[:, t, :],
                            start=(t == 0), stop=(t == NDCH - 1))
                        if i == NI - 1 and (lsub * NDCH + t) % 2 == 1:
                            drain(1)
                    nc.scalar.copy(out=v_sb[:, i * 4 + lsub, :], in_=ps)
                if i == NI - 1:
                    drain_tail()

        # ---------------- Phase C+D: attention + fused out-projection -------
        with tc.tile_pool(name="pc2", bufs=2) as pc2, \
             tc.tile_pool(name="pcw", bufs=1) as pcw:
            # W_out cached for all 4 i-blocks; the DMA overlaps the first
            # attention pairs (its SBUF region frees as phase B retires)
            wo_sb = pcw.tile([128, NDCH, D], BF16)
            nc.gpsimd.dma_start(out=wo_sb, in_=wo_d.ap())

            ctxTs = {}

            def emit_pv(i, h, expS, rb):
                ctxT = pctx.tile([128, 4, 512], BF16, tag=f"ctx{h}")
                for ec in range(4):
                    ps = psA.tile([128, 512], F32, tag="acc")
                    for j in range(NJ):
                        nc.tensor.matmul(
                            ps, lhsT=v_sb[:, j, ec * 128:(ec + 1) * 128],
                            rhs=expS[:, j, :],
                            start=(j == 0), stop=(j == NJ - 1))
                        # front-loaded (1 per 2): the denominator adds of the
                        # next pair resolve mid-segment
                        if (ec * NJ + j) % 2 == 1:
                            drain(1)
                        # its reduce+reciprocal tail goes late, after the
                        # vector adds have certainly finished
                        if ec == 3 and j == 8:
                            drain_tail()
                    nc.vector.tensor_mul(ctxT[:, ec, :], ps, rb)
                ctxTs[h] = ctxT

            def emit_outproj(i):
                for dm in range(4):
                    for lsub in range(4):
                        ps = psA.tile([128, 512], F32, tag="acc")
                        for h in range(NHQ):
                            for ec in range(4):
                                t = h * 4 + ec
                                nc.tensor.matmul(
                                    ps,
                                    lhsT=ctxTs[h][:, ec,
                                                  lsub * 128:(lsub + 1) * 128],
                                    rhs=wo_sb[:, t, dm * 512:(dm + 1) * 512],
                                    start=(t == 0), stop=(t == 15))
                        ost = pc2.tile([128, 512], F32, tag="ost")
                        nc.scalar.copy(out=ost, in_=ps)
                        l0 = i * 512 + lsub * 128
                        nc.sync.dma_start(
                            out=out_d.ap()[l0:l0 + 128,
                                           dm * 512:(dm + 1) * 512],
                            in_=ost)

            pairs = [(i, h) for i in range(NI) for h in range(NHQ)]
            handles = {(0, 0): first_handle}
            for k, (i, h) in enumerate(pairs):
                if k + 1 < len(pairs):
                    handles[pairs[k + 1]] = queue_s(*pairs[k + 1])
                emit_pv(i, h, *handles.pop((i, h)))
                if h == NHQ - 1:
                    emit_outproj(i)
            assert not sq

    nc.compile()
    _NC_CACHE["nc"] = nc
    return nc


def make_core_inputs(x, W_attn, W_out):
    """Split full inputs into 8 per-core input maps (core = b*4 + g).

    All tensors are pre-arranged into the exact SBUF layout (partition
    dim first) so device DMAs are large contiguous packets.
    """
    Q_DIM = 2048
    K_DIM = 512
    scale = np.float32(1.0 / math.sqrt(DK))
    bf = ml_dtypes.bfloat16

    # rope tables, mirroring the fp32 reference computation
    inv_freq = (np.float32(1.0) /
                (np.float32(10000.0) **
                 (np.arange(0, DK, 2, dtype=np.float32) / np.float32(DK))))
    freqs = np.arange(L, dtype=np.float32)[:, None] * inv_freq[None, :]  # [L,64]
    ang = np.concatenate([freqs, freqs], axis=-1)  # [L, 128]
    cosT = np.ascontiguousarray(np.cos(ang).T.astype(np.float32))  # [128, L]
    sinT = np.ascontiguousarray(np.sin(ang).T.astype(np.float32))

    # x^T -> [NI, 128, NDCH, 512]: block i, partition p, chunk t, q
    xTb = []
    for b in range(2):
        xt = x[b].T.astype(bf)                       # [D, L]
        xt = xt.reshape(NDCH, 128, NI, 512).transpose(2, 1, 0, 3)
        xTb.append(np.ascontiguousarray(xt))

    in_maps = []
    for core in range(8):
        b, g = divmod(core, 4)
        wq = (W_attn[:, 512 * g:512 * (g + 1)] * scale)
        wk = W_attn[:, Q_DIM + 128 * g:Q_DIM + 128 * (g + 1)]
        wqk = np.concatenate([wq, wk], axis=1).astype(bf)        # [D, 640]
        wqk = np.ascontiguousarray(
            wqk.reshape(NDCH, 128, 5, 128).transpose(2, 1, 0, 3))
        wv = W_attn[:, Q_DIM + K_DIM + 512 * g:
                    Q_DIM + K_DIM + 512 * (g + 1)].astype(bf)    # [D, 512]
        wv = np.ascontiguousarray(wv.reshape(NDCH, 128, DV).transpose(1, 0, 2))
        wo = W_out[2048 * g:2048 * (g + 1), :].astype(bf)        # [2048, D]
        wo = np.ascontiguousarray(wo.reshape(NDCH, 128, D).transpose(1, 0, 2))
        in_maps.append({
            "xt": xTb[b],
            "wqk": wqk,
            "wv": wv,
            "wo": wo,
            "cost": cosT,
            "sint": sinT,
        })
    return in_maps


def kernel(x, W_attn, W_out, b_out, _trace=False, _trace_cores=None):
    x = np.asarray(x)
    W_attn = np.asarray(W_attn)
    W_out = np.asarray(W_out)
    b_out = np.asarray(b_out)
    nc = build_nc()
    in_maps = make_core_inputs(x, W_attn, W_out)
    res = run_bass_kernel_spmd(
        nc, in_maps, core_ids=list(range(8)),
        trace=_trace, trace_cores=_trace_cores)
    parts = [res.results[c]["out"] for c in range(8)]
    out = np.empty((2, L, D), dtype=np.float32)
    for b in range(2):
        acc = parts[4 * b].astype(np.float32)
        for g in range(1, 4):
            acc = acc + parts[4 * b + g]
        out[b] = acc + b_out[None, :].astype(np.float32)
    if _trace:
        kernel._last_results = res
    return out


# revision 31
# speedup vs baseline: 1.0177x; 1.0021x over previous
"""GQA kernel for trn2, 8 cores: DP over batch (2) x TP over kv-head groups (4).

Each core computes, for its (batch b, kv-group g):
  - qkv projection for its 4 q-heads + 1 kv-head (q pre-scaled by 1/sqrt(dk))
  - RoPE on q/k
  - full (non-causal) attention for the 4 q-heads vs its kv-head
  - partial out-projection with its 2048 rows of W_out
Host sums the 4 per-group partials per batch and adds bias.

Matmul operands are bf16 (PE runs fp32 at 1/4 rate; bf16 is full rate).
Accumulation and softmax statistics stay fp32.

Perf notes:
  - all inputs are pre-arranged on the host into the exact SBUF layout
    (partition-dim first) so every DMA moves large contiguous packets
  - x is pre-transposed on the host; no PE cycles are spent transposing
  - softmax denominator off the tensor engine: vector adds (one behind
    each exp) + gpsimd partition all-reduce + vector reciprocal
  - score matmuls of the next pair are interleaved into the PV matmul
    stream (1 per 4) so the scalar engine's exp rate never stalls the PE
  - W_out cached in SBUF once (loaded during the B->C transition)
  - warmup matmuls ramp the PE clock out of its low p-state while the
    first input DMAs are in flight

Self-contained: hardcodes all shapes. kernel(**inputs) -> np.ndarray.
"""

import math
from contextlib import ExitStack

import numpy as np
import ml_dtypes

import concourse.bass as bass
import concourse.bass_isa as bass_isa
import concourse.bacc as bacc
import concourse.tile as tile
import concourse.mybir as mybir
from concourse.bass_utils import run_bass_kernel_spmd

F32 = mybir.dt.float32
F32R = mybir.dt.float32r
BF16 = mybir.dt.bfloat16
L = 2048          # sequence length
D = 2048          # d_model
DK = 128          # head dim (q/k)
DV = 512          # head dim (v)
NHQ = 4           # q heads per core
NI = 4            # query chunks of 512
NJ = 16           # key chunks of 128
NDCH = 16         # d_model chunks of 128

_NC_CACHE = {}


def build_nc():
    if "nc" in _NC_CACHE:
        return _NC_CACHE["nc"]
    nc = bacc.Bacc("TRN2", target_bir_lowering=False, debug=False)

    # all inputs pre-arranged to SBUF layout on the host
    xT_d = nc.dram_tensor("xt", [NI, 128, NDCH, 512], BF16, kind="ExternalInput")
    wqk_d = nc.dram_tensor("wqk", [5, 128, NDCH, 128], BF16, kind="ExternalInput")
    wv_d = nc.dram_tensor("wv", [128, NDCH, DV], BF16, kind="ExternalInput")
    wo_d = nc.dram_tensor("wo", [128, NDCH, D], BF16, kind="ExternalInput")
    cos_d = nc.dram_tensor("cost", [DK, L], F32, kind="ExternalInput")
    sin_d = nc.dram_tensor("sint", [DK, L], F32, kind="ExternalInput")
    out_d = nc.dram_tensor("out", [L, D], F32, kind="ExternalOutput")

    EXP = mybir.ActivationFunctionType.Exp

    with ExitStack() as ctx:
        tc = ctx.enter_context(tile.TileContext(nc))
        # long-lived pools
        persist = ctx.enter_context(tc.tile_pool(name="persist", bufs=1))
        pexp = ctx.enter_context(tc.tile_pool(name="pexp", bufs=2))
        pden = ctx.enter_context(tc.tile_pool(name="pden", bufs=2))
        pctx = ctx.enter_context(tc.tile_pool(name="pctx", bufs=1))
        psS = ctx.enter_context(tc.tile_pool(name="psS", bufs=5, space="PSUM"))
        psA = ctx.enter_context(tc.tile_pool(name="psA", bufs=3, space="PSUM"))

        qT = persist.tile([128, NHQ, L], BF16)      # [dk, h, pos]
        kT = persist.tile([128, L], BF16)           # [dk, pos]
        v_sb = persist.tile([128, NJ, DV], BF16)    # [key_in_chunk, key_chunk, e]
        ones_b = persist.tile([128, 128], BF16)     # all-ones: denom reduce
        nc.vector.memset(ones_b, 1.0)

        # warmup: ramp the PE out of its low p-state while input DMAs fly
        warm = persist.tile([128, 512], BF16)
        nc.vector.memset(warm, 0.0)
        for w in range(18):
            wps = psS.tile([128, 512], F32, tag="stream")
            nc.tensor.matmul(wps, lhsT=warm[:, 0:128], rhs=warm)

        # ---- S/exp/denominator step queue (absorbed into other segments) ----
        sq = []
        # denominator tail steps (partition-reduce matmul + reciprocal),
        # drained late in the next PV segment, after the vector adds finish
        tailq = []

        def queue_s(i, h):
            """Queue the 16 score-matmul+exp+denom steps for pair (i, h)."""
            isl = slice(i * 512, (i + 1) * 512)
            expS = pexp.tile([128, NJ, 512], BF16, tag="expS")
            acc = pden.tile([128, 512], F32, tag="dacc")
            accb = pden.tile([128, 512], BF16, tag="daccb")
            rb = pden.tile([128, 512], F32, tag="rb")

            def mk(j):
                def f():
                    ps = psS.tile([128, 512], F32, tag="stream")
                    nc.tensor.matmul(ps, lhsT=kT[:, j * 128:(j + 1) * 128],
                                     rhs=qT[:, h, isl])
                    nc.scalar.activation(out=expS[:, j, :], in_=ps, func=EXP)
                    if j == 1:
                        nc.vector.tensor_add(acc, expS[:, 0, :], expS[:, 1, :])
                    elif j == NJ - 1:
                        # final add rounds once to bf16 so the reduce matmul
                        # below runs at full bf16 rate
                        nc.vector.tensor_add(accb, acc, expS[:, j, :])
                    elif j > 1:
                        nc.vector.tensor_add(acc, acc, expS[:, j, :])
                return f

            def tail():
                # all-ones stationary: one full-rate matmul both sums over
                # partitions and replicates the denominator to all partitions
                dps = psS.tile([128, 512], F32, tag="stream")
                nc.tensor.matmul(dps, lhsT=ones_b, rhs=accb)
                nc.vector.reciprocal_approx_fast(out=rb, in_=dps)

            sq.extend(mk(j) for j in range(NJ))
            tailq.append(tail)
            return expS, rb

        def drain(n=1):
            for _ in range(n):
                if sq:
                    sq.pop(0)()

        def drain_tail():
            while tailq:
                tailq.pop(0)()

        # ---------------- Phase B: qkv projection + rope ----------------
        with tc.tile_pool(name="pb1", bufs=1) as pb1, \
             tc.tile_pool(name="pbx", bufs=2) as pbx, \
             tc.tile_pool(name="pb2", bufs=2) as pb2:
            wqk_sb = pb1.tile([128, 5, NDCH, 128], BF16)
            wv_sb = pb1.tile([128, NDCH, DV], BF16)
            cosT = pb1.tile([128, L], F32)
            sinT = pb1.tile([128, L], F32)

            xTs = [pbx.tile([128, NDCH, 512], BF16, tag="xT", name=f"xT{i}")
                   for i in range(NI)]
            # DMA issue order == first-use order. sync queue: xT chunks.
            # gpsimd queue: weights + rope tables.
            for tq in range(4):
                nc.sync.dma_start(out=xTs[0][:, 4 * tq:4 * tq + 4, :],
                                  in_=xT_d.ap()[0][:, 4 * tq:4 * tq + 4, :])
            nc.gpsimd.dma_start(out=wqk_sb[:, 0], in_=wqk_d.ap()[0])
            nc.gpsimd.dma_start(out=wqk_sb[:, 1], in_=wqk_d.ap()[1])
            nc.gpsimd.dma_start(out=cosT[:, 0:512], in_=cos_d.ap()[:, 0:512])
            nc.gpsimd.dma_start(out=sinT[:, 0:512], in_=sin_d.ap()[:, 0:512])
            for c in range(2, 5):
                nc.gpsimd.dma_start(out=wqk_sb[:, c], in_=wqk_d.ap()[c])
            nc.gpsimd.dma_start(out=cosT[:, 512:], in_=cos_d.ap()[:, 512:])
            nc.gpsimd.dma_start(out=sinT[:, 512:], in_=sin_d.ap()[:, 512:])
            nc.gpsimd.dma_start(out=wv_sb, in_=wv_d.ap())
            for i in range(1, NI):
                nc.sync.dma_start(out=xTs[i], in_=xT_d.ap()[i])

            for i in range(NI):
                isl = slice(i * 512, (i + 1) * 512)
                xT = xTs[i]
                # q/k projection + rope (c = 0..3 q heads, c = 4 is k)
                for c in range(5):
                    ps = psA.tile([128, 512], F32, tag="acc")
                    for t in range(NDCH):
                        nc.tensor.matmul(
                            ps, lhsT=wqk_sb[:, c, t, :],
                            rhs=xT[:, t, :],
                            start=(t == 0), stop=(t == NDCH - 1))
                    dest = qT[:, c, isl] if c < NHQ else kT[:, isl]
                    cs = cosT[:, isl]
                    sn = sinT[:, isl]
                    tmp = pb2.tile([128, 512], F32, tag="rope")
                    nc.vector.tensor_mul(tmp[0:64, :], ps[64:128, :], sn[0:64, :])
                    nc.vector.tensor_mul(tmp[64:128, :], ps[0:64, :], sn[64:128, :])
                    tmp2 = pb2.tile([128, 512], F32, tag="rope2")
                    nc.vector.tensor_mul(tmp2, ps, cs)
                    nc.vector.tensor_sub(dest[0:64, :], tmp2[0:64, :], tmp[0:64, :])
                    nc.vector.tensor_add(dest[64:128, :], tmp2[64:128, :],
                                         tmp[64:128, :])

                # v projection; the i==3 block absorbs the score matmuls of
                # the first attention pair (kT is complete at that point)
                if i == NI - 1:
                    first_handle = queue_s(0, 0)
                for lsub in range(4):
                    ps = psA.tile([128, 512], F32, tag="acc")
                    for t in range(NDCH):
                        nc.tensor.matmul(
                            ps, lhsT=xT[:, t, lsub * 128:(lsub + 1) * 128],
                            rhs=wv_sb[:, t, :],
                            start=(t == 0), stop=(t == NDCH - 1))
                        if i == NI - 1 and (lsub * NDCH + t) % 2 == 1:
                            drain(1)
                    nc.scalar.copy(out=v_sb[:, i * 4 + lsub, :], in_=ps)
                if i == NI - 1:
                    drain_tail()

        # ---------------- Phase C+D: attention + fused out-projection -------
        with tc.tile_pool(name="pc2", bufs=2) as pc2, \
             tc.tile_pool(name="pcw", bufs=1) as pcw:
            # W_out cached for all 4 i-blocks; the DMA overlaps the first
            # attention pairs (its SBUF region frees as phase B retires)
            wo_sb = pcw.tile([128, NDCH, D], BF16)
            nc.gpsimd.dma_start(out=wo_sb, in_=wo_d.ap())

            ctxTs = {}

            def emit_pv(i, h, expS, rb):
                ctxT = pctx.tile([128, 4, 512], BF16, tag=f"ctx{h}")
                for ec in range(4):
                    ps = psA.tile([128, 512], F32, tag="acc")
                    for j in range(NJ):
                        nc.tensor.matmul(
                            ps, lhsT=v_sb[:, j, ec * 128:(ec + 1) * 128],
                            rhs=expS[:, j, :],
                            start=(j == 0), stop=(j == NJ - 1))
                        # front-loaded (1 per 2): the denominator adds of the
                        # next pair resolve mid-segment
                        if (ec * NJ + j) % 2 == 1:
                            drain(1)
                        # its reduce+reciprocal tail goes late, after the
                        # vector adds have certainly finished
                        if ec == 3 and j == 8:
                            drain_tail()
                    nc.vector.tensor_mul(ctxT[:, ec, :], ps, rb)
                ctxTs[h] = ctxT

            def emit_outproj(i):
                for dm in range(4):
                    for lsub in range(4):
                        ps = psA.tile([128, 512], F32, tag="acc")
                        for h in range(NHQ):
                            for ec in range(4):
                                t = h * 4 + ec
                                nc.tensor.matmul(
                                    ps,
                                    lhsT=ctxTs[h][:, ec,
                                                  lsub * 128:(lsub + 1) * 128],
                                    rhs=wo_sb[:, t, dm * 512:(dm + 1) * 512],
                                    start=(t == 0), stop=(t == 15))
                        ost = pc2.tile([128, 512], F32, tag="ost")
                        nc.scalar.copy(out=ost, in_=ps)
                        l0 = i * 512 + lsub * 128
                        nc.sync.dma_start(
                            out=out_d.ap()[l0:l0 + 128,
                                           dm * 512:(dm + 1) * 512],
                            in_=ost)

            pairs = [(i, h) for i in range(NI) for h in range(NHQ)]
            handles = {(0, 0): first_handle}
            for k, (i, h) in enumerate(pairs):
                if k + 1 < len(pairs):
                    handles[pairs[k + 1]] = queue_s(*pairs[k + 1])
                emit_pv(i, h, *handles.pop((i, h)))
                if h == NHQ - 1:
                    emit_outproj(i)
            assert not sq

    nc.compile()
    _NC_CACHE["nc"] = nc
    return nc


def make_core_inputs(x, W_attn, W_out):
    """Split full inputs into 8 per-core input maps (core = b*4 + g).

    All tensors are pre-arranged into the exact SBUF layout (partition
    dim first) so device DMAs are large contiguous packets.
    """
    Q_DIM = 2048
    K_DIM = 512
    scale = np.float32(1.0 / math.sqrt(DK))
    bf = ml_dtypes.bfloat16

    # rope tables, mirroring the fp32 reference computation
    inv_freq = (np.float32(1.0) /
                (np.float32(10000.0) **
                 (np.arange(0, DK, 2, dtype=np.float32) / np.float32(DK))))
    freqs = np.arange(L, dtype=np.float32)[:, None] * inv_freq[None, :]  # [L,64]
    ang = np.concatenate([freqs, freqs], axis=-1)  # [L, 128]
    cosT = np.ascontiguousarray(np.cos(ang).T.astype(np.float32))  # [128, L]
    sinT = np.ascontiguousarray(np.sin(ang).T.astype(np.float32))

    # x^T -> [NI, 128, NDCH, 512]: block i, partition p, chunk t, q
    xTb = []
    for b in range(2):
        xt = x[b].T.astype(bf)                       # [D, L]
        xt = xt.reshape(NDCH, 128, NI, 512).transpose(2, 1, 0, 3)
        xTb.append(np.ascontiguousarray(xt))

    in_maps = []
    for core in range(8):
        b, g = divmod(core, 4)
        wq = (W_attn[:, 512 * g:512 * (g + 1)] * scale)
        wk = W_attn[:, Q_DIM + 128 * g:Q_DIM + 128 * (g + 1)]
        wqk = np.concatenate([wq, wk], axis=1).astype(bf)        # [D, 640]
        wqk = np.ascontiguousarray(
            wqk.reshape(NDCH, 128, 5, 128).transpose(2, 1, 0, 3))
        wv = W_attn[:, Q_DIM + K_DIM + 512 * g:
                    Q_DIM + K_DIM + 512 * (g + 1)].astype(bf)    # [D, 512]
        wv = np.ascontiguousarray(wv.reshape(NDCH, 128, DV).transpose(1, 0, 2))
        wo = W_out[2048 * g:2048 * (g + 1), :].astype(bf)        # [2048, D]
        wo = np.ascontiguousarray(wo.reshape(NDCH, 128, D).transpose(1, 0, 2))
        in_maps.append({
            "xt": xTb[b],
            "wqk": wqk,
            "wv": wv,
            "wo": wo,
            "cost": cosT,
            "sint": sinT,
        })
    return in_maps


def kernel(x, W_attn, W_out, b_out, _trace=False, _trace_cores=None):
    x = np.asarray(x)
    W_attn = np.asarray(W_attn)
    W_out = np.asarray(W_out)
    b_out = np.asarray(b_out)
    nc = build_nc()
    in_maps = make_core_inputs(x, W_attn, W_out)
    res = run_bass_kernel_spmd(
        nc, in_maps, core_ids=list(range(8)),
        trace=_trace, trace_cores=_trace_cores)
    parts = [res.results[c]["out"] for c in range(8)]
    out = np.empty((2, L, D), dtype=np.float32)
    for b in range(2):
        acc = parts[4 * b].astype(np.float32)
        for g in range(1, 4):
            acc = acc + parts[4 * b + g]
        out[b] = acc + b_out[None, :].astype(np.float32)
    if _trace:
        kernel._last_results = res
    return out
